# revision 19
# baseline (speedup 1.0000x reference)
"""MoE (ExpertPool) kernel for Trainium2, 8 NeuronCores.

Strategy: host computes the (tiny) router: logits = x@Wr+br, top-2
selection, softmax combine weights. Every expert is then STRIPED across
all 8 cores: expert e's token list is split into 8 nearly-equal slices,
and core i processes slice i of every expert. The device program is one
SPMD template of E=8 chunks (chunk e = expert e, size ceil(c_e/8)), so
each core does a near-identical ~T*top_k/8 tokens of work (perfect load
balance; <=7 zero-gate pad tokens per expert) while streaming all 8
experts' weights (~153 GB/s, split over the sync+gpsimd DMA rings —
well under the ~358 GB/s/core HBM budget). Host scatter-adds the
per-(expert, slice) outputs back ("combine").

Device kernel: everything feature-major (features on SBUF partitions,
tokens on the free dim), bf16 operands (rel-err ~4e-3 vs the 2e-2 gate;
bf16 enables the PE's Fast Weight Load path so the 128-col LDWEIGHTS
(~27ns vs ~190ns for f32r, which is excluded from FWL) fully hides under
the matmul streams — f32r weight loads were ~95us of exposed PE time).
Chunk columns split into <=512-wide groups, one single-bank PSUM
accumulator each. GELU+bias fuse into ScalarE activations reading PSUM;
gating is a DVE multiply against a partition-broadcast gate row; x tiles
prefetch a chunk ahead on the scalar ring; y stores ride the vector
ring. Output is written feature-major, transposed on the host during the
combine.
"""

import numpy as np

# Problem dims (hardcoded per spec: nn_ExpertPool_8366596292698)
B, S, D, E, I = 8, 2048, 768, 8, 3072
H = I // 2
T = B * S
P = 128
KD, KI, KH = D // P, I // P, H // P  # 6, 24, 12
N_CORES = 8

_PROGRAM_CACHE: dict = {}
LAST_RESULTS = None  # BassKernelResults of the most recent run (for test harness)


def _col_groups(nc_tokens):
    """Column groups (start, len), each <=512 wide (single-bank PSUM),
    split evenly so no group is disproportionately narrow."""
    n = -(-nc_tokens // 512)
    base = nc_tokens // n
    extra = nc_tokens - base * n
    out = []
    s = 0
    for i in range(n):
        ln = base + (1 if i < extra else 0)
        out.append((s, ln))
        s += ln
    return out


def _build_program(chunks, has_b1, has_b2, has_b3):
    from contextlib import ExitStack

    import concourse.bacc as bacc
    import concourse.bass as bass
    import concourse.mybir as mybir
    import concourse.tile as tile

    f32 = mybir.dt.float32
    bf16 = mybir.dt.bfloat16
    GELU = mybir.ActivationFunctionType.Gelu

    C = sum(chunks)
    NCH = len(chunks)

    nc = bacc.Bacc(
        "TRN2",
        target_bir_lowering=False,
        debug=False,
        enable_asserts=False,
        num_devices=N_CORES,
    )

    # host-pretiled layouts: every DMA below reads/writes one fully
    # contiguous block. Weights carry a leading per-chunk (= per-expert)
    # slot dimension.
    xT = nc.dram_tensor("xTt", [D * C], bf16, kind="ExternalInput").ap()
    w1 = nc.dram_tensor("w1t", [NCH, KI, P, KD * P], bf16, kind="ExternalInput").ap()
    w2 = nc.dram_tensor("w2t", [NCH, KH, P, KI * P], bf16, kind="ExternalInput").ap()
    w3 = nc.dram_tensor("w3t", [NCH, KD, P, KH * P], bf16, kind="ExternalInput").ap()
    gate = nc.dram_tensor("gate", [C], f32, kind="ExternalInput").ap()
    b1 = b2 = b3 = None
    if has_b1:
        b1 = nc.dram_tensor("b1t", [NCH, P, KI], f32, kind="ExternalInput").ap()
    if has_b2:
        b2 = nc.dram_tensor("b2t", [NCH, P, KH], f32, kind="ExternalInput").ap()
    if has_b3:
        b3 = nc.dram_tensor("b3t", [NCH, P, KD], f32, kind="ExternalInput").ap()
    yT = nc.dram_tensor("yTt", [D * C], f32, kind="ExternalOutput").ap()

    with tile.TileContext(nc) as tc, ExitStack() as ctx:
        bpool = ctx.enter_context(tc.tile_pool(name="bias", bufs=2))
        xpool = ctx.enter_context(tc.tile_pool(name="x", bufs=2))
        h1pool = ctx.enter_context(tc.tile_pool(name="h1", bufs=1))
        h2pool = ctx.enter_context(tc.tile_pool(name="h2", bufs=1))
        w1pool = ctx.enter_context(tc.tile_pool(name="w1p", bufs=12))
        w2pool = ctx.enter_context(tc.tile_pool(name="w2p", bufs=8))
        w3pool = ctx.enter_context(tc.tile_pool(name="w3p", bufs=6))
        ypool = ctx.enter_context(tc.tile_pool(name="y", bufs=2))
        gpool = ctx.enter_context(tc.tile_pool(name="g", bufs=2))
        pspool = ctx.enter_context(
            tc.tile_pool(name="ps", bufs=8, space=bass.MemorySpace.PSUM)
        )

        # weight panels alternate between the sync and gpsimd rings so the
        # sustained ~153 GB/s weight stream never saturates one ring
        wrings = [nc.sync, nc.gpsimd]
        wring_i = [0]

        def wdma(dst, src):
            wrings[wring_i[0] % 2].dma_start(dst, src)
            wring_i[0] += 1

        # chunk start offsets
        bases = []
        b_ = 0
        for s in chunks:
            bases.append(b_)
            b_ += s

        def load_chunk_inputs(ci):
            """DMA this chunk's token activations + gate row + biases on
            the scalar ring, keeping sync+gpsimd free to prefetch weight
            panels (chunk 0's first panels are on the startup critical
            path)."""
            Nc = chunks[ci]
            base = bases[ci]
            x_sb = []
            for k in range(KD):
                xk = xpool.tile([P, Nc], bf16, tag=f"x{k}")
                off = (base * D) + k * P * Nc
                # chunk 0: odd x tiles ride the sync ring (right behind the
                # pre-issued first W1 panel) so the ramp isn't serialized
                # on one ring
                eng = (nc.scalar, nc.sync)[k % 2] if ci == 0 else nc.scalar
                eng.dma_start(
                    xk[:], xT[off : off + P * Nc].rearrange("(p f) -> p f", f=Nc)
                )
                x_sb.append(xk)
            g_bc = gpool.tile([P, Nc], f32, tag="gbc")
            nc.scalar.dma_start(
                g_bc[:],
                gate[base : base + Nc].unsqueeze(0).partition_broadcast(P).squeeze(1),
            )
            bs = []
            for has, dram, kk, tg in (
                (has_b1, b1, KI, "b1"),
                (has_b2, b2, KH, "b2"),
                (has_b3, b3, KD, "b3"),
            ):
                if has:
                    bt = bpool.tile([P, kk], f32, tag=tg)
                    nc.scalar.dma_start(bt[:], dram[ci])
                    bs.append(bt)
                else:
                    bs.append(None)
            return x_sb, g_bc, bs

        # the very first W1 panel gates all compute: issue it first on the
        # sync ring, before chunk 0's x tiles queue up
        w1p_first = w1pool.tile([P, KD * P], bf16, tag="w1p")
        nc.sync.dma_start(w1p_first[:], w1[0, 0])
        wring_i[0] = 1  # next weight panel starts on the gpsimd ring

        pending = load_chunk_inputs(0)
        for ci, Nc in enumerate(chunks):
            base = bases[ci]
            cgs = _col_groups(Nc)
            x_sb, g_bc, (b1_sb, b2_sb, b3_sb) = pending

            # ---- L1: h1 = gelu(x @ W1 + b1), feature-major [I, Nc] ----
            h1_sb = []
            for m in range(KI):
                if ci == 0 and m == 0:
                    w1p = w1p_first
                else:
                    w1p = w1pool.tile([P, KD * P], bf16, tag="w1p")
                    if ci == 0:
                        # ramp: all three rings carry W1 so W2 can start
                        # flowing on the weight rings sooner
                        (nc.sync, nc.gpsimd, nc.scalar)[m % 3].dma_start(
                            w1p[:], w1[ci, m]
                        )
                    else:
                        wdma(w1p[:], w1[ci, m])
                h1m = h1pool.tile([P, Nc], bf16, tag=f"h1_{m}")
                # groups interleave inside the k-loop: consecutive matmuls
                # share the stationary weight panel, so its LDWEIGHTS is
                # loaded once per k instead of once per (k, group)
                pss = [
                    pspool.tile([P, cn], f32, tag="ps", name=f"ps{gi}")
                    for gi, (_, cn) in enumerate(cgs)
                ]
                for k in range(KD):
                    for ps, (cs, cn) in zip(pss, cgs):
                        nc.tensor.matmul(
                            ps[:],
                            lhsT=w1p[:, k * P : (k + 1) * P],
                            rhs=x_sb[k][:, cs : cs + cn],
                            start=(k == 0),
                            stop=(k == KD - 1),
                        )
                for ps, (cs, cn) in zip(pss, cgs):
                    nc.scalar.activation(
                        h1m[:, cs : cs + cn],
                        ps[:],
                        GELU,
                        bias=(b1_sb[:, m : m + 1] if has_b1 else 0.0),
                    )
                h1_sb.append(h1m)

            # prefetch next chunk's activations; lands during L1/L2
            if ci + 1 < len(chunks):
                pending = load_chunk_inputs(ci + 1)

            # ---- L2: h2 = gelu(h1 @ W2 + b2), feature-major [H, Nc] ----
            h2_sb = []
            for m in range(KH):
                w2p = w2pool.tile([P, KI * P], bf16, tag="w2p")
                if ci == 0 and m < 3:
                    # ramp: first W2 panels bypass the W1-laden weight rings
                    nc.scalar.dma_start(w2p[:], w2[ci, m])
                else:
                    wdma(w2p[:], w2[ci, m])
                h2m = h2pool.tile([P, Nc], bf16, tag=f"h2_{m}")
                pss = [
                    pspool.tile([P, cn], f32, tag="ps", name=f"ps{gi}")
                    for gi, (_, cn) in enumerate(cgs)
                ]
                for k in range(KI):
                    for ps, (cs, cn) in zip(pss, cgs):
                        nc.tensor.matmul(
                            ps[:],
                            lhsT=w2p[:, k * P : (k + 1) * P],
                            rhs=h1_sb[k][:, cs : cs + cn],
                            start=(k == 0),
                            stop=(k == KI - 1),
                        )
                for ps, (cs, cn) in zip(pss, cgs):
                    nc.scalar.activation(
                        h2m[:, cs : cs + cn],
                        ps[:],
                        GELU,
                        bias=(b2_sb[:, m : m + 1] if has_b2 else 0.0),
                    )
                h2_sb.append(h2m)

            # ---- L3: y = (h2 @ W3 + b3) * gate, feature-major [D, Nc] ----
            for m in range(KD):
                w3p = w3pool.tile([P, KH * P], bf16, tag="w3p")
                wdma(w3p[:], w3[ci, m])
                y_sb = ypool.tile([P, Nc], f32, tag="y")
                pss = [
                    pspool.tile([P, cn], f32, tag="ps", name=f"ps{gi}")
                    for gi, (_, cn) in enumerate(cgs)
                ]
                for k in range(KH):
                    for ps, (cs, cn) in zip(pss, cgs):
                        nc.tensor.matmul(
                            ps[:],
                            lhsT=w3p[:, k * P : (k + 1) * P],
                            rhs=h2_sb[k][:, cs : cs + cn],
                            start=(k == 0),
                            stop=(k == KH - 1),
                        )
                for ps, (cs, cn) in zip(pss, cgs):
                    if has_b3:
                        nc.vector.tensor_scalar_add(
                            y_sb[:, cs : cs + cn], ps[:], b3_sb[:, m : m + 1]
                        )
                        nc.vector.tensor_mul(
                            y_sb[:, cs : cs + cn],
                            y_sb[:, cs : cs + cn],
                            g_bc[:, cs : cs + cn],
                        )
                    else:
                        nc.vector.tensor_mul(
                            y_sb[:, cs : cs + cn], ps[:], g_bc[:, cs : cs + cn]
                        )
                yoff = (base * D) + m * P * Nc
                nc.scalar.dma_start(
                    yT[yoff : yoff + P * Nc].rearrange("(p f) -> p f", f=Nc), y_sb[:]
                )

    nc.compile()
    return nc


def _route(x, Wr, br, top_k):
    """Host router: fp32 logits, stable top-k, softmax weights."""
    xt = np.ascontiguousarray(x.reshape(T, D), dtype=np.float32)
    logits = (xt @ np.asarray(Wr, np.float32)) + np.asarray(br, np.float32)
    k = int(top_k)
    # descending by value, ties -> lower index (matches jax.lax.top_k)
    order = np.argsort(-logits, axis=1, kind="stable")[:, :k]  # [T, k]
    vals = np.take_along_axis(logits, order, axis=1)
    vmax = vals.max(axis=1, keepdims=True)
    ex = np.exp(vals - vmax)
    wts = (ex / ex.sum(axis=1, keepdims=True)).astype(np.float32)
    return xt, order, wts


def kernel(x, Wr, br, W1, b1, W2, b2, W3, b3, top_k):
    global LAST_RESULTS
    import os

    import ml_dtypes
    from concourse import bass_utils

    bf = ml_dtypes.bfloat16

    x = np.asarray(x)
    out_dtype = x.dtype
    xt, sel, wts = _route(x, Wr, br, top_k)

    W1 = np.asarray(W1, np.float32)
    W2 = np.asarray(W2, np.float32)
    W3 = np.asarray(W3, np.float32)
    b1 = np.asarray(b1, np.float32)
    b2 = np.asarray(b2, np.float32)
    b3 = np.asarray(b3, np.float32)

    # token lists per expert
    idx_e = []
    gate_e = []
    for e in range(E):
        rows, cols = np.nonzero(sel == e)
        idx_e.append(rows)
        gate_e.append(wts[rows, cols])
    counts = np.array([len(i) for i in idx_e])

    # stripe: one chunk per expert on every core, core i takes slice i of
    # each expert's token list; largest chunk first (the ramp overlaps the
    # most work), smallest last (shortest post-PE drain tail)
    experts = sorted(
        (e for e in range(E) if counts[e]), key=lambda e: -counts[e]
    )
    chunks = tuple(int(-(-counts[e] // N_CORES)) for e in experts)

    has_b1 = bool(np.any(b1))
    has_b2 = bool(np.any(b2))
    has_b3 = bool(np.any(b3))

    key = (chunks, has_b1, has_b2, has_b3)
    if key not in _PROGRAM_CACHE:
        _PROGRAM_CACHE[key] = _build_program(list(chunks), has_b1, has_b2, has_b3)
    nc = _PROGRAM_CACHE[key]

    C = sum(chunks)
    bases = []
    b_ = 0
    for s in chunks:
        bases.append(b_)
        b_ += s

    def tile_w(w, km):
        """[K, M] -> [km_panels, 128, K] panel-contiguous bf16 layout."""
        K, M = w.shape
        # panel m: element (p, a*128+f) = w[a*128+p, m*128+f]
        v = w.reshape(K // P, P, km, P)  # [a, p, m, f]
        return np.ascontiguousarray(v.transpose(2, 1, 0, 3).astype(bf)).reshape(
            km, P, K
        )

    # weight slots are identical on every core: one shared array each
    w1t = np.stack([tile_w(W1[e], KI) for e in experts])
    w2t = np.stack([tile_w(W2[e], KH) for e in experts])
    w3t = np.stack([tile_w(W3[e], KD) for e in experts])
    if has_b1:
        b1t = np.stack([np.ascontiguousarray(b1[e].reshape(KI, P).T) for e in experts])
    if has_b2:
        b2t = np.stack([np.ascontiguousarray(b2[e].reshape(KH, P).T) for e in experts])
    if has_b3:
        b3t = np.stack([np.ascontiguousarray(b3[e].reshape(KD, P).T) for e in experts])

    xtb = xt.astype(bf)

    # per-(core, chunk) slice bounds into each expert's token list
    def slice_bounds(ci, i):
        n = int(counts[experts[ci]])
        s = chunks[ci]
        lo = min(i * s, n)
        hi = min(lo + s, n)
        return lo, hi

    in_maps = []
    for i in range(N_CORES):
        xp = np.zeros((D * C,), bf)
        g = np.zeros((C,), np.float32)
        for ci in range(len(chunks)):
            e = experts[ci]
            Nc = chunks[ci]
            base = bases[ci]
            lo, hi = slice_bounds(ci, i)
            n = hi - lo
            blk = np.zeros((D, Nc), bf)
            if n:
                blk[:, :n] = xtb[idx_e[e][lo:hi]].T
                g[base : base + n] = gate_e[e][lo:hi]
            xp[base * D : (base + Nc) * D] = blk.reshape(-1)
        m = {"xTt": xp, "w1t": w1t, "w2t": w2t, "w3t": w3t, "gate": g}
        if has_b1:
            m["b1t"] = b1t
        if has_b2:
            m["b2t"] = b2t
        if has_b3:
            m["b3t"] = b3t
        in_maps.append(m)

    trace_cores = None
    if os.environ.get("BASS_TRACE") and not os.environ.get(
        "BASS_PERFETTO_PROFILE_ALL_CORES"
    ):
        trace_cores = [0]

    res = bass_utils.run_bass_kernel_spmd(
        nc,
        in_maps,
        core_ids=list(range(N_CORES)),
        trace_cores=trace_cores,
    )
    LAST_RESULTS = res

    out = np.zeros((T, D), np.float32)
    # accumulate expert-ascending to match reference summation order
    for ci in range(len(chunks)):
        e = experts[ci]
        Nc = chunks[ci]
        base = bases[ci]
        for i in range(N_CORES):
            lo, hi = slice_bounds(ci, i)
            n = hi - lo
            if n:
                blk = res.results[i]["yTt"][base * D : (base + Nc) * D]
                yTe = blk.reshape(D, Nc)
                out[idx_e[e][lo:hi]] += yTe[:, :n].T
    return out.reshape(B, S, D).astype(out_dtype, copy=False)


# revision 21
# speedup vs baseline: 1.0091x; 1.0091x over previous
"""MoE (ExpertPool) kernel for Trainium2, 8 NeuronCores.

Strategy: host computes the (tiny) router: logits = x@Wr+br, top-2
selection, softmax combine weights. Every expert is then STRIPED across
all 8 cores: expert e's token list is split into 8 nearly-equal slices,
and core i processes slice i of every expert. The device program is one
SPMD template of E=8 chunks (chunk e = expert e, size ceil(c_e/8)), so
each core does a near-identical ~T*top_k/8 tokens of work (perfect load
balance; <=7 zero-gate pad tokens per expert) while streaming all 8
experts' weights (~153 GB/s, split over the sync+gpsimd DMA rings —
well under the ~358 GB/s/core HBM budget). Host scatter-adds the
per-(expert, slice) outputs back ("combine").

Device kernel: everything feature-major (features on SBUF partitions,
tokens on the free dim), bf16 operands (rel-err ~4e-3 vs the 2e-2 gate;
bf16 enables the PE's Fast Weight Load path so the 128-col LDWEIGHTS
(~27ns vs ~190ns for f32r, which is excluded from FWL) fully hides under
the matmul streams — f32r weight loads were ~95us of exposed PE time).
Chunk columns split into <=512-wide groups, one single-bank PSUM
accumulator each, interleaved inside the k-loop. GELU+bias fuse into
ScalarE activations reading PSUM; gating is a DVE multiply against a
partition-broadcast gate row; x tiles prefetch a chunk ahead on the
scalar ring, which also carries gate rows and y stores. Output is
written feature-major, transposed on the host during the combine.
Measured: ~910 us on the slowest core (vs 1024 us for the f32r
expert-per-core predecessor); ~94.5% of the 78.6 TF/s PE column-stream
floor for the 4099-token/core stripe.
"""

import numpy as np

# Problem dims (hardcoded per spec: nn_ExpertPool_8366596292698)
B, S, D, E, I = 8, 2048, 768, 8, 3072
H = I // 2
T = B * S
P = 128
KD, KI, KH = D // P, I // P, H // P  # 6, 24, 12
N_CORES = 8

_PROGRAM_CACHE: dict = {}
LAST_RESULTS = None  # BassKernelResults of the most recent run (for test harness)


def _col_groups(nc_tokens):
    """Column groups (start, len), each <=512 wide (single-bank PSUM),
    split evenly so no group is disproportionately narrow."""
    n = -(-nc_tokens // 512)
    base = nc_tokens // n
    extra = nc_tokens - base * n
    out = []
    s = 0
    for i in range(n):
        ln = base + (1 if i < extra else 0)
        out.append((s, ln))
        s += ln
    return out


def _build_program(chunks, has_b1, has_b2, has_b3):
    from contextlib import ExitStack

    import concourse.bacc as bacc
    import concourse.bass as bass
    import concourse.mybir as mybir
    import concourse.tile as tile

    f32 = mybir.dt.float32
    bf16 = mybir.dt.bfloat16
    GELU = mybir.ActivationFunctionType.Gelu

    C = sum(chunks)
    NCH = len(chunks)

    nc = bacc.Bacc(
        "TRN2",
        target_bir_lowering=False,
        debug=False,
        enable_asserts=False,
        num_devices=N_CORES,
    )

    # host-pretiled layouts: every DMA below reads/writes one fully
    # contiguous block. Weights carry a leading per-chunk (= per-expert)
    # slot dimension.
    xT = nc.dram_tensor("xTt", [D * C], bf16, kind="ExternalInput").ap()
    w1 = nc.dram_tensor("w1t", [NCH, KI, P, KD * P], bf16, kind="ExternalInput").ap()
    w2 = nc.dram_tensor("w2t", [NCH, KH, P, KI * P], bf16, kind="ExternalInput").ap()
    w3 = nc.dram_tensor("w3t", [NCH, KD, P, KH * P], bf16, kind="ExternalInput").ap()
    gate = nc.dram_tensor("gate", [C], f32, kind="ExternalInput").ap()
    b1 = b2 = b3 = None
    if has_b1:
        b1 = nc.dram_tensor("b1t", [NCH, P, KI], f32, kind="ExternalInput").ap()
    if has_b2:
        b2 = nc.dram_tensor("b2t", [NCH, P, KH], f32, kind="ExternalInput").ap()
    if has_b3:
        b3 = nc.dram_tensor("b3t", [NCH, P, KD], f32, kind="ExternalInput").ap()
    yT = nc.dram_tensor("yTt", [D * C], f32, kind="ExternalOutput").ap()

    with tile.TileContext(nc) as tc, ExitStack() as ctx:
        bpool = ctx.enter_context(tc.tile_pool(name="bias", bufs=2))
        xpool = ctx.enter_context(tc.tile_pool(name="x", bufs=2))
        h1pool = ctx.enter_context(tc.tile_pool(name="h1", bufs=1))
        h2pool = ctx.enter_context(tc.tile_pool(name="h2", bufs=1))
        w1pool = ctx.enter_context(tc.tile_pool(name="w1p", bufs=12))
        w2pool = ctx.enter_context(tc.tile_pool(name="w2p", bufs=8))
        w3pool = ctx.enter_context(tc.tile_pool(name="w3p", bufs=6))
        ypool = ctx.enter_context(tc.tile_pool(name="y", bufs=2))
        gpool = ctx.enter_context(tc.tile_pool(name="g", bufs=2))
        pspool = ctx.enter_context(
            tc.tile_pool(name="ps", bufs=8, space=bass.MemorySpace.PSUM)
        )

        # weight panels alternate between the sync and gpsimd rings so the
        # sustained ~153 GB/s weight stream never saturates one ring
        wrings = [nc.sync, nc.gpsimd]
        wring_i = [0]

        def wdma(dst, src):
            wrings[wring_i[0] % 2].dma_start(dst, src)
            wring_i[0] += 1

        # chunk start offsets
        bases = []
        b_ = 0
        for s in chunks:
            bases.append(b_)
            b_ += s

        def load_chunk_inputs(ci):
            """DMA this chunk's token activations + gate row + biases on
            the scalar ring, keeping sync+gpsimd free to prefetch weight
            panels (chunk 0's first panels are on the startup critical
            path)."""
            Nc = chunks[ci]
            base = bases[ci]
            x_sb = []
            for k in range(KD):
                xk = xpool.tile([P, Nc], bf16, tag=f"x{k}")
                off = (base * D) + k * P * Nc
                # chunk 0: x tiles spread over all three rings (behind the
                # pre-issued first W1 panel) so the ramp isn't serialized
                # on one ring
                eng = (nc.scalar, nc.sync, nc.gpsimd)[k % 3] if ci == 0 else nc.scalar
                eng.dma_start(
                    xk[:], xT[off : off + P * Nc].rearrange("(p f) -> p f", f=Nc)
                )
                x_sb.append(xk)
            g_bc = gpool.tile([P, Nc], f32, tag="gbc")
            nc.scalar.dma_start(
                g_bc[:],
                gate[base : base + Nc].unsqueeze(0).partition_broadcast(P).squeeze(1),
            )
            bs = []
            for has, dram, kk, tg in (
                (has_b1, b1, KI, "b1"),
                (has_b2, b2, KH, "b2"),
                (has_b3, b3, KD, "b3"),
            ):
                if has:
                    bt = bpool.tile([P, kk], f32, tag=tg)
                    nc.scalar.dma_start(bt[:], dram[ci])
                    bs.append(bt)
                else:
                    bs.append(None)
            return x_sb, g_bc, bs

        # the very first W1 panel gates all compute: issue it first on the
        # sync ring, before chunk 0's x tiles queue up
        w1p_first = w1pool.tile([P, KD * P], bf16, tag="w1p")
        nc.sync.dma_start(w1p_first[:], w1[0, 0])
        wring_i[0] = 1  # next weight panel starts on the gpsimd ring

        pending = load_chunk_inputs(0)
        for ci, Nc in enumerate(chunks):
            base = bases[ci]
            cgs = _col_groups(Nc)
            x_sb, g_bc, (b1_sb, b2_sb, b3_sb) = pending

            # ---- L1: h1 = gelu(x @ W1 + b1), feature-major [I, Nc] ----
            h1_sb = []
            for m in range(KI):
                if ci == 0 and m == 0:
                    w1p = w1p_first
                else:
                    w1p = w1pool.tile([P, KD * P], bf16, tag="w1p")
                    if ci == 0:
                        # ramp: all three rings carry W1 so W2 can start
                        # flowing on the weight rings sooner
                        (nc.sync, nc.gpsimd, nc.scalar)[m % 3].dma_start(
                            w1p[:], w1[ci, m]
                        )
                    else:
                        wdma(w1p[:], w1[ci, m])
                h1m = h1pool.tile([P, Nc], bf16, tag=f"h1_{m}")
                # groups interleave inside the k-loop: consecutive matmuls
                # share the stationary weight panel, so its LDWEIGHTS is
                # loaded once per k instead of once per (k, group)
                pss = [
                    pspool.tile([P, cn], f32, tag="ps", name=f"ps{gi}")
                    for gi, (_, cn) in enumerate(cgs)
                ]
                for k in range(KD):
                    for ps, (cs, cn) in zip(pss, cgs):
                        nc.tensor.matmul(
                            ps[:],
                            lhsT=w1p[:, k * P : (k + 1) * P],
                            rhs=x_sb[k][:, cs : cs + cn],
                            start=(k == 0),
                            stop=(k == KD - 1),
                        )
                for ps, (cs, cn) in zip(pss, cgs):
                    nc.scalar.activation(
                        h1m[:, cs : cs + cn],
                        ps[:],
                        GELU,
                        bias=(b1_sb[:, m : m + 1] if has_b1 else 0.0),
                    )
                h1_sb.append(h1m)

            # prefetch next chunk's activations; lands during L1/L2
            if ci + 1 < len(chunks):
                pending = load_chunk_inputs(ci + 1)

            # ---- L2: h2 = gelu(h1 @ W2 + b2), feature-major [H, Nc] ----
            h2_sb = []
            for m in range(KH):
                w2p = w2pool.tile([P, KI * P], bf16, tag="w2p")
                if ci == 0 and m < 3:
                    # ramp: first W2 panels bypass the W1-laden weight rings
                    nc.scalar.dma_start(w2p[:], w2[ci, m])
                else:
                    wdma(w2p[:], w2[ci, m])
                h2m = h2pool.tile([P, Nc], bf16, tag=f"h2_{m}")
                pss = [
                    pspool.tile([P, cn], f32, tag="ps", name=f"ps{gi}")
                    for gi, (_, cn) in enumerate(cgs)
                ]
                for k in range(KI):
                    for ps, (cs, cn) in zip(pss, cgs):
                        nc.tensor.matmul(
                            ps[:],
                            lhsT=w2p[:, k * P : (k + 1) * P],
                            rhs=h1_sb[k][:, cs : cs + cn],
                            start=(k == 0),
                            stop=(k == KI - 1),
                        )
                for ps, (cs, cn) in zip(pss, cgs):
                    nc.scalar.activation(
                        h2m[:, cs : cs + cn],
                        ps[:],
                        GELU,
                        bias=(b2_sb[:, m : m + 1] if has_b2 else 0.0),
                    )
                h2_sb.append(h2m)

            # ---- L3: y = (h2 @ W3 + b3) * gate, feature-major [D, Nc] ----
            for m in range(KD):
                w3p = w3pool.tile([P, KH * P], bf16, tag="w3p")
                wdma(w3p[:], w3[ci, m])
                y_sb = ypool.tile([P, Nc], f32, tag="y")
                pss = [
                    pspool.tile([P, cn], f32, tag="ps", name=f"ps{gi}")
                    for gi, (_, cn) in enumerate(cgs)
                ]
                for k in range(KH):
                    for ps, (cs, cn) in zip(pss, cgs):
                        nc.tensor.matmul(
                            ps[:],
                            lhsT=w3p[:, k * P : (k + 1) * P],
                            rhs=h2_sb[k][:, cs : cs + cn],
                            start=(k == 0),
                            stop=(k == KH - 1),
                        )
                for ps, (cs, cn) in zip(pss, cgs):
                    if has_b3:
                        nc.vector.tensor_scalar_add(
                            y_sb[:, cs : cs + cn], ps[:], b3_sb[:, m : m + 1]
                        )
                        nc.vector.tensor_mul(
                            y_sb[:, cs : cs + cn],
                            y_sb[:, cs : cs + cn],
                            g_bc[:, cs : cs + cn],
                        )
                    else:
                        nc.vector.tensor_mul(
                            y_sb[:, cs : cs + cn], ps[:], g_bc[:, cs : cs + cn]
                        )
                yoff = (base * D) + m * P * Nc
                nc.scalar.dma_start(
                    yT[yoff : yoff + P * Nc].rearrange("(p f) -> p f", f=Nc), y_sb[:]
                )

    nc.compile()
    return nc


def _route(x, Wr, br, top_k):
    """Host router: fp32 logits, stable top-k, softmax weights."""
    xt = np.ascontiguousarray(x.reshape(T, D), dtype=np.float32)
    logits = (xt @ np.asarray(Wr, np.float32)) + np.asarray(br, np.float32)
    k = int(top_k)
    # descending by value, ties -> lower index (matches jax.lax.top_k)
    order = np.argsort(-logits, axis=1, kind="stable")[:, :k]  # [T, k]
    vals = np.take_along_axis(logits, order, axis=1)
    vmax = vals.max(axis=1, keepdims=True)
    ex = np.exp(vals - vmax)
    wts = (ex / ex.sum(axis=1, keepdims=True)).astype(np.float32)
    return xt, order, wts


def kernel(x, Wr, br, W1, b1, W2, b2, W3, b3, top_k):
    global LAST_RESULTS
    import os

    import ml_dtypes
    from concourse import bass_utils

    bf = ml_dtypes.bfloat16

    x = np.asarray(x)
    out_dtype = x.dtype
    xt, sel, wts = _route(x, Wr, br, top_k)

    W1 = np.asarray(W1, np.float32)
    W2 = np.asarray(W2, np.float32)
    W3 = np.asarray(W3, np.float32)
    b1 = np.asarray(b1, np.float32)
    b2 = np.asarray(b2, np.float32)
    b3 = np.asarray(b3, np.float32)

    # token lists per expert
    idx_e = []
    gate_e = []
    for e in range(E):
        rows, cols = np.nonzero(sel == e)
        idx_e.append(rows)
        gate_e.append(wts[rows, cols])
    counts = np.array([len(i) for i in idx_e])

    # stripe: one chunk per expert on every core, core i takes slice i of
    # each expert's token list; largest chunk first (the ramp overlaps the
    # most work), smallest last (shortest post-PE drain tail)
    experts = sorted(
        (e for e in range(E) if counts[e]), key=lambda e: -counts[e]
    )
    chunks = tuple(int(-(-counts[e] // N_CORES)) for e in experts)

    has_b1 = bool(np.any(b1))
    has_b2 = bool(np.any(b2))
    has_b3 = bool(np.any(b3))

    key = (chunks, has_b1, has_b2, has_b3)
    if key not in _PROGRAM_CACHE:
        _PROGRAM_CACHE[key] = _build_program(list(chunks), has_b1, has_b2, has_b3)
    nc = _PROGRAM_CACHE[key]

    C = sum(chunks)
    bases = []
    b_ = 0
    for s in chunks:
        bases.append(b_)
        b_ += s

    def tile_w(w, km):
        """[K, M] -> [km_panels, 128, K] panel-contiguous bf16 layout."""
        K, M = w.shape
        # panel m: element (p, a*128+f) = w[a*128+p, m*128+f]
        v = w.reshape(K // P, P, km, P)  # [a, p, m, f]
        return np.ascontiguousarray(v.transpose(2, 1, 0, 3).astype(bf)).reshape(
            km, P, K
        )

    # weight slots are identical on every core: one shared array each
    w1t = np.stack([tile_w(W1[e], KI) for e in experts])
    w2t = np.stack([tile_w(W2[e], KH) for e in experts])
    w3t = np.stack([tile_w(W3[e], KD) for e in experts])
    if has_b1:
        b1t = np.stack([np.ascontiguousarray(b1[e].reshape(KI, P).T) for e in experts])
    if has_b2:
        b2t = np.stack([np.ascontiguousarray(b2[e].reshape(KH, P).T) for e in experts])
    if has_b3:
        b3t = np.stack([np.ascontiguousarray(b3[e].reshape(KD, P).T) for e in experts])

    xtb = xt.astype(bf)

    # per-(core, chunk) slice bounds into each expert's token list
    def slice_bounds(ci, i):
        n = int(counts[experts[ci]])
        s = chunks[ci]
        lo = min(i * s, n)
        hi = min(lo + s, n)
        return lo, hi

    in_maps = []
    for i in range(N_CORES):
        xp = np.zeros((D * C,), bf)
        g = np.zeros((C,), np.float32)
        for ci in range(len(chunks)):
            e = experts[ci]
            Nc = chunks[ci]
            base = bases[ci]
            lo, hi = slice_bounds(ci, i)
            n = hi - lo
            blk = np.zeros((D, Nc), bf)
            if n:
                blk[:, :n] = xtb[idx_e[e][lo:hi]].T
                g[base : base + n] = gate_e[e][lo:hi]
            xp[base * D : (base + Nc) * D] = blk.reshape(-1)
        m = {"xTt": xp, "w1t": w1t, "w2t": w2t, "w3t": w3t, "gate": g}
        if has_b1:
            m["b1t"] = b1t
        if has_b2:
            m["b2t"] = b2t
        if has_b3:
            m["b3t"] = b3t
        in_maps.append(m)

    trace_cores = None
    if os.environ.get("BASS_TRACE") and not os.environ.get(
        "BASS_PERFETTO_PROFILE_ALL_CORES"
    ):
        trace_cores = [0]

    res = bass_utils.run_bass_kernel_spmd(
        nc,
        in_maps,
        core_ids=list(range(N_CORES)),
        trace_cores=trace_cores,
    )
    LAST_RESULTS = res

    out = np.zeros((T, D), np.float32)
    # accumulate expert-ascending to match reference summation order
    for ci in range(len(chunks)):
        e = experts[ci]
        Nc = chunks[ci]
        base = bases[ci]
        for i in range(N_CORES):
            lo, hi = slice_bounds(ci, i)
            n = hi - lo
            if n:
                blk = res.results[i]["yTt"][base * D : (base + Nc) * D]
                yTe = blk.reshape(D, Nc)
                out[idx_e[e][lo:hi]] += yTe[:, :n].T
    return out.reshape(B, S, D).astype(out_dtype, copy=False)
